# revision 21
# baseline (speedup 1.0000x reference)
"""Trainium2 Bass kernel for nn_CT_37821482009425 (snntorch Leaky LIF scan).

Reference semantics (bitwise-matched):
    T = clip(t, 1, 5); x = roll(inp, roll_amount, axis=2)
    per step: reset = (mem > T); mem = 0.95*mem + x_t - reset*T; spk = (mem > T)
Output: spikes (1024, 1, 224, 224) float32 in {0, 1}.

Distribution: pure data parallelism — batch 1024 -> 8 cores x 128 (the SBUF
partition dim). Host prep per core: apply the roll and transpose to
time-major so each timestep's H=224 vector is contiguous per partition.

Per-core compute (vector engine, per time step, all stock DVE ops whose
rounding matches the reference exactly):
    v      = scalar_tensor_tensor(mem[t-1], 0.95, x_t, mult, add)
    mem_t  = tensor_tensor(v, r[t-1], subtract)
    r_t    = tensor_scalar(mem_t, T, T, is_gt, mult)        # in {0, T}
r_t doubles as the reset feedback and the spike record (spk = r != 0 on host).
Input DMA (sync engine) and output DMA (scalar engine) run in 8-step slices,
double-buffered against compute in 32-step chunk buffers.
"""

import numpy as np
import concourse.bass as bass
import concourse.mybir as mybir
from concourse.bass_utils import run_bass_kernel_spmd

BETA = 0.95
B, CH = 1024, 224
N_CORES = 8
PB = B // N_CORES  # 128 batches per core = partition dim
H = CH  # per-step vector length (contiguous, time-major)
W = CH  # time steps
WC = 32  # chunk size (SBUF buffer granularity)
SUB = 2  # DMA slice granularity (steps)
N_CHUNK = W // WC
SUBS_PER_CHUNK = WC // SUB
N_SUB = W // SUB

_Alu = mybir.AluOpType

_cache = {}


def _build(T: float):
    nc = bass.Bass(trn_type="TRN2")
    x_d = nc.dram_tensor("x", [PB, W * H], mybir.dt.float32, kind="ExternalInput")
    r_d = nc.dram_tensor("r", [PB, W * H], mybir.dt.float32, kind="ExternalOutput")

    with (
        nc.sbuf_tensor("xt0", [PB, WC * H], mybir.dt.float32) as xt0,
        nc.sbuf_tensor("xt1", [PB, WC * H], mybir.dt.float32) as xt1,
        nc.sbuf_tensor("rt0", [PB, WC * H], mybir.dt.float32) as rt0,
        nc.sbuf_tensor("rt1", [PB, WC * H], mybir.dt.float32) as rt1,
        nc.sbuf_tensor("mcol", [PB, 2 * H], mybir.dt.float32) as mcol,
        nc.sbuf_tensor("vcol", [PB, H], mybir.dt.float32) as vcol,
        nc.semaphore() as in_sem,
        nc.semaphore() as v_sem,
        nc.semaphore() as out_sem,
        nc.Block() as block,
    ):
        xb = [xt0, xt1]
        rb = [rt0, rt1]

        # v_sem: vector increments once per completed SUB-slice (28 total).

        @block.sync
        def _(sync):
            # input DMA, one 8-step slice at a time
            for sb in range(N_SUB):
                c, sl = divmod(sb, SUBS_PER_CHUNK)
                if c >= 2:
                    # xt[c%2] slice sl is free once chunk c-2's compute has
                    # fully consumed that slice (vector bumps v_sem per slice)
                    sync.wait_ge(v_sem, (c - 2) * SUBS_PER_CHUNK + sl + 1)
                sync.dma_start(
                    xb[c % 2][:, sl * SUB * H : (sl + 1) * SUB * H],
                    x_d[:, sb * SUB * H : (sb + 1) * SUB * H],
                ).then_inc(in_sem, 16)

        @block.scalar
        def _(scalar):
            for sb in range(N_SUB):
                c, sl = divmod(sb, SUBS_PER_CHUNK)
                scalar.wait_ge(v_sem, sb + 1)
                scalar.dma_start(
                    r_d[:, sb * SUB * H : (sb + 1) * SUB * H],
                    rb[c % 2][:, sl * SUB * H : (sl + 1) * SUB * H],
                ).then_inc(out_sem, 16)

        @block.vector
        def _(vector):
            for sb in range(N_SUB):
                c, sl = divmod(sb, SUBS_PER_CHUNK)
                vector.wait_ge(in_sem, 16 * (sb + 1))
                if sl == 0 and c >= 2:
                    # rt[c%2] free once all its out-DMA slices (chunk c-2)
                    # completed
                    vector.wait_ge(out_sem, 16 * (c - 1) * SUBS_PER_CHUNK)
                xt, rt = xb[c % 2], rb[c % 2]
                for tl in range(sl * SUB, (sl + 1) * SUB):
                    t = c * WC + tl
                    xcol = xt[:, tl * H : (tl + 1) * H]
                    rcol = rt[:, tl * H : (tl + 1) * H]
                    mc = mcol[:, (t % 2) * H : (t % 2 + 1) * H]
                    if t == 0:
                        nc.vector.tensor_copy(mc, xcol)
                    else:
                        mp = mcol[:, ((t - 1) % 2) * H : ((t - 1) % 2 + 1) * H]
                        if tl == 0:
                            rprev = rb[(c - 1) % 2][:, (WC - 1) * H :]
                        else:
                            rprev = rt[:, (tl - 1) * H : tl * H]
                        nc.vector.scalar_tensor_tensor(
                            vcol[:], mp, BETA, xcol, _Alu.mult, _Alu.add
                        )
                        nc.vector.tensor_tensor(mc, vcol[:], rprev, _Alu.subtract)
                    ts = nc.vector.tensor_scalar(
                        rcol, mc, T, T, _Alu.is_gt, _Alu.mult
                    )
                    if tl % SUB == SUB - 1:
                        ts.then_inc(v_sem, 1)

    return nc


def kernel(inp: np.ndarray, t: np.ndarray, roll_amount) -> np.ndarray:
    T = float(
        np.clip(np.float32(np.asarray(t).reshape(-1)[0]), np.float32(1.0),
                np.float32(5.0))
    )
    roll = int(np.asarray(roll_amount)) % W

    key = (T,)
    if key not in _cache:
        _cache[key] = _build(T)
    nc = _cache[key]

    inp = np.asarray(inp, dtype=np.float32).reshape(B, CH, CH)
    in_maps = []
    for c in range(N_CORES):
        shard = inp[c * PB : (c + 1) * PB]  # (128, H, W)
        shard = np.roll(shard, roll, axis=2)
        # time-major: (128, W, H) contiguous
        x_tm = np.ascontiguousarray(shard.transpose(0, 2, 1)).reshape(PB, W * H)
        in_maps.append({"x": x_tm})

    res = run_bass_kernel_spmd(nc, in_maps, core_ids=list(range(N_CORES)))

    out = np.empty((B, 1, CH, CH), dtype=np.float32)
    for c in range(N_CORES):
        r = res.results[c]["r"].reshape(PB, W, H)  # (b, w, h)
        out[c * PB : (c + 1) * PB, 0] = (r != 0).transpose(0, 2, 1)
    return out


# revision 23
# speedup vs baseline: 1.0026x; 1.0026x over previous
"""Trainium2 Bass kernel for nn_CT_37821482009425 (snntorch Leaky LIF scan).

Reference semantics (bitwise-matched):
    T = clip(t, 1, 5); x = roll(inp, roll_amount, axis=2)
    per step: reset = (mem > T); mem = 0.95*mem + x_t - reset*T; spk = (mem > T)
Output: spikes (1024, 1, 224, 224) float32 in {0, 1}.

Distribution: pure data parallelism — batch 1024 -> 8 cores x 128 (the SBUF
partition dim). Host prep per core: apply the roll and transpose to
time-major so each timestep's H=224 vector is contiguous per partition.

Per-core compute (DVE/vector engine; all ops round bitwise-identically to
the reference, so the output matches exactly):
    v     = scalar_tensor_tensor(mem[t-1], 0.95, x_t, mult, add)
    mem_t = tensor_tensor(v, r[t-1], subtract)          # r in {0, T}
    r_t   = tensor_scalar(mem_t, T, T, is_gt, mult)     # spike record + feedback
The recurrence is a strict serial chain (224 steps x 3 dependent DVE ops),
which is the binding resource; DMA is fully hidden under it. The DVE is the
only engine that can run this loop here: GPSIMD/Pool compute does not pass
neuronxcc codegen via this toolchain, the Activation engine has no two-tensor
ops and its per-op latency (~378ns vs ts2's 183ns) would lengthen the serial
cycle, and PE fp32 matmul runs at 4 cycles/row. Offloading the spike op
cross-engine was measured (TimelineSim) to add ~200ns/step of semaphore
round-trip, so everything stays on the DVE.

DMA schedule: input streams in 8-step chunks through a 4-deep SBUF ring
(chunk DMAs are issued NB chunks ahead so the ~4.8us DMA round-trip latency
stays off the compute critical path); the first chunk is fetched in 2-step
slices so compute starts ~2.8us in. Output drains in 2-step slices (1-step,
via the then-idle SP queue, at the very end) so only ~1.7us of store latency
trails the last compute op.

NOTE: reading x in place as mem_0 (skipping the t=0 tensor_copy) caused
INTERMITTENT spike corruption on real hardware (~50% of runs, first
divergence in early chunks) — keep the copy. Any change to the DMA/compute
semaphore structure must be stress-tested >=6 consecutive device runs.
"""

import numpy as np
import concourse.bass as bass
import concourse.mybir as mybir
from concourse.bass_utils import run_bass_kernel_spmd

BETA = 0.95
B, CH = 1024, 224
N_CORES = 8
PB = B // N_CORES  # 128 batches per core = partition dim
H = CH  # per-step vector length (contiguous, time-major)
W = CH  # time steps
WC = 8  # chunk size (SBUF buffer + DMA granularity)
N_CHUNK = W // WC
NB = 4  # buffer ring depth (chunks in flight)
FIRST_SLICES = (2, 2, 2, 2)  # first-chunk DMA slice sizes (steps)
FS = len(FIRST_SLICES)  # number of first-chunk slices
HD = 224  # all chains on DVE (Pool/gpsimd compute does not compile via this toolchain)

# map: step index -> number of first-chunk DMA slices that must have landed
FIRST_STEP_WAITS = {}
_s = 0
for _i, _sl in enumerate(FIRST_SLICES):
    FIRST_STEP_WAITS[_s] = _i + 1
    _s += _sl

_Alu = mybir.AluOpType

_cache = {}


def _build(T: float):
    nc = bass.Bass(trn_type="TRN2")
    x_d = nc.dram_tensor("x", [PB, W * H], mybir.dt.float32, kind="ExternalInput")
    r_d = nc.dram_tensor("r", [PB, W * H], mybir.dt.float32, kind="ExternalOutput")

    import contextlib

    with contextlib.ExitStack() as stack:
        xb = [
            stack.enter_context(
                nc.sbuf_tensor(f"xt{i}", [PB, WC * H], mybir.dt.float32)
            )
            for i in range(NB)
        ]
        rb = [
            stack.enter_context(
                nc.sbuf_tensor(f"rt{i}", [PB, WC * H], mybir.dt.float32)
            )
            for i in range(NB)
        ]
        mcol = stack.enter_context(
            nc.sbuf_tensor("mcol", [PB, 2 * H], mybir.dt.float32)
        )
        vcol = stack.enter_context(nc.sbuf_tensor("vcol", [PB, H], mybir.dt.float32))
        in_sem = stack.enter_context(nc.semaphore())
        dve_sem = stack.enter_context(nc.semaphore())
        pool_sem = stack.enter_context(nc.semaphore())
        out_sem = stack.enter_context(nc.semaphore())
        block = stack.enter_context(nc.Block())

        def emit_out(eng, s_lo, s_hi):
            # 2-step output slices (1-step at the very end) so the final
            # store latency after the last compute op is minimal
            s = s_lo
            while s < s_hi:
                sl = 1 if s >= W - 2 else 2
                c, tl = divmod(s, WC)
                e = s + sl - 1  # last step in this slice
                # dve_sem increments after every odd step, plus once at W-2
                eng.wait_ge(dve_sem, (e + 1) // 2 + (1 if e >= W - 2 else 0))
                eng.dma_start(
                    r_d[:, s * H : (s + sl) * H],
                    rb[c % NB][:, tl * H : (tl + sl) * H],
                ).then_inc(out_sem, 16)
                s += sl

        @block.sync
        def _(sync):
            # input DMA, one chunk at a time (first chunk in FS slices)
            for c in range(N_CHUNK):
                if c >= NB:
                    # xb[c%NB] free once chunk c-NB's compute consumed it
                    # (wait one chunk further than strictly needed: slack
                    # against semaphore/write-visibility timing on real HW)
                    sync.wait_ge(dve_sem, min(c - NB + 2, N_CHUNK) * (WC // 2))
                if c == 0:
                    s = 0
                    for sl in FIRST_SLICES:
                        sync.dma_start(
                            xb[0][:, s * H : (s + sl) * H],
                            x_d[:, s * H : (s + sl) * H],
                        ).then_inc(in_sem, 16)
                        s += sl
                else:
                    sync.dma_start(
                        xb[c % NB][:], x_d[:, c * WC * H : (c + 1) * WC * H]
                    ).then_inc(in_sem, 16)
            # the last chunk's stores go via SP (idle once inputs are done;
            # shorter DGE->DMA delay than Act shortens the kernel tail)
            emit_out(sync, W - WC, W)

        @block.scalar
        def _(scalar):
            emit_out(scalar, 0, W - WC)

        def compute(eng, sem, lo, hi):
            for c in range(N_CHUNK):
                if c > 0:
                    eng.wait_ge(in_sem, 16 * (FS - 1 + c + 1))
                if c >= NB:
                    # rb[c%NB] free once chunk c-NB's out-DMA completed
                    eng.wait_ge(out_sem, 16 * (c - NB + 1) * (WC // 2))
                xt, rt = xb[c % NB], rb[c % NB]
                for tl in range(WC):
                    t = c * WC + tl
                    if c == 0 and t in FIRST_STEP_WAITS:
                        eng.wait_ge(in_sem, 16 * FIRST_STEP_WAITS[t])
                    xcol = xt[:, tl * H + lo : tl * H + hi]
                    rcol = rt[:, tl * H + lo : tl * H + hi]
                    mc = mcol[:, (t % 2) * H + lo : (t % 2) * H + hi]
                    vc = vcol[:, lo:hi]
                    if t == 0:
                        eng.tensor_copy(mc, xcol)
                    else:
                        mp = mcol[
                            :, ((t - 1) % 2) * H + lo : ((t - 1) % 2) * H + hi
                        ]
                        if tl == 0:
                            rprev = rb[(c - 1) % NB][
                                :, (WC - 1) * H + lo : (WC - 1) * H + hi
                            ]
                        else:
                            rprev = rt[:, (tl - 1) * H + lo : (tl - 1) * H + hi]
                        eng.scalar_tensor_tensor(vc, mp, BETA, xcol, _Alu.mult, _Alu.add)
                        eng.tensor_tensor(mc, vc, rprev, _Alu.subtract)
                    ts = eng.tensor_scalar(rcol, mc, T, T, _Alu.is_gt, _Alu.mult)
                    if t % 2 == 1 or t >= W - 2:
                        ts.then_inc(sem, 1)

        @block.vector
        def _(vector):
            compute(nc.vector, dve_sem, 0, HD)

    return nc


def kernel(inp: np.ndarray, t: np.ndarray, roll_amount) -> np.ndarray:
    T = float(
        np.clip(np.float32(np.asarray(t).reshape(-1)[0]), np.float32(1.0),
                np.float32(5.0))
    )
    roll = int(np.asarray(roll_amount)) % W

    key = (T,)
    if key not in _cache:
        _cache[key] = _build(T)
    nc = _cache[key]

    inp = np.asarray(inp, dtype=np.float32).reshape(B, CH, CH)
    in_maps = []
    for c in range(N_CORES):
        shard = inp[c * PB : (c + 1) * PB]  # (128, H, W)
        shard = np.roll(shard, roll, axis=2)
        # time-major: (128, W, H) contiguous
        x_tm = np.ascontiguousarray(shard.transpose(0, 2, 1)).reshape(PB, W * H)
        in_maps.append({"x": x_tm})

    res = run_bass_kernel_spmd(nc, in_maps, core_ids=list(range(N_CORES)))

    out = np.empty((B, 1, CH, CH), dtype=np.float32)
    for c in range(N_CORES):
        r = res.results[c]["r"].reshape(PB, W, H)  # (b, w, h) in {0, T}
        out[c * PB : (c + 1) * PB, 0] = (r != 0).transpose(0, 2, 1)
    return out
